# revision 1
# baseline (speedup 1.0000x reference)
"""Bass/Trainium2 kernel for nn_Attn (dot+affect attention over encoder outputs).

Computation (per batch b):
  e[b, l] = h[b] . enc[l, b]  +  (h[b] @ affect) . emb[l, b]
  out[b, 0, :] = softmax(e[b, :])

Strategy: data-parallel over batch (8 batches per core on 8 cores). The host
packs enc and emb into one [L, BLOC*(H+A)] tensor per core so one wide VectorE
multiply per 4.2MB slab + per-batch free-dim reductions (1 VectorE
tensor_reduce + 7 ScalarE activation-accumulates) compute the attention
energies in two elementwise passes, both under the DMA streaming rate
(memory-bound regime). h@affect runs on the TensorEngine; the h-broadcast
arrives pre-replicated via DMA. Scores are laid out o-major and transposed
incrementally per slab so the final softmax (mask matmuls for partition-group
sums/broadcasts, fused Exp+accumulate) is a short tail.
"""

import numpy as np

import concourse.bass as bass
import concourse.tile as tile
from concourse import bacc, mybir
from concourse.bass_utils import run_bass_kernel_spmd

F32 = mybir.dt.float32
L, B, H, A = 2048, 64, 1024, 3
NCORES = 8
BLOC = B // NCORES          # batches per core
HE = H + A                  # extended hidden width (dot + affect features)
P = 128                     # SBUF partitions / l-tile height


def build_nc(l_total: int = L):
    no = l_total // P       # number of l-tiles
    cols = BLOC * no        # score columns, o-major: c = o*BLOC + b

    nc = bacc.Bacc("TRN2", target_bir_lowering=False, debug=False)

    enc_d = nc.dram_tensor("enc", [l_total, BLOC * HE], F32, kind="ExternalInput")
    hid_d = nc.dram_tensor("hid", [BLOC, H], F32, kind="ExternalInput")
    aff_d = nc.dram_tensor("aff", [H, A], F32, kind="ExternalInput")
    ident_d = nc.dram_tensor("ident", [P, P], F32, kind="ExternalInput")
    ones_d = nc.dram_tensor("ones_", [1, P], F32, kind="ExternalInput")
    bm_d = nc.dram_tensor("bm", [cols, BLOC], F32, kind="ExternalInput")
    bmT_d = nc.dram_tensor("bmT", [BLOC, cols], F32, kind="ExternalInput")
    nbmT_d = nc.dram_tensor("nbmT", [BLOC, cols], F32, kind="ExternalInput")
    sel_d = nc.dram_tensor("sel", [BLOC, BLOC * P], F32, kind="ExternalInput")
    hbx_d = nc.dram_tensor("hbx", [P, BLOC * HE], F32, kind="ExternalInput")
    out_d = nc.dram_tensor("out", [cols, P], F32, kind="ExternalOutput")

    add = mybir.AluOpType.add
    amax = mybir.AluOpType.max
    AX = mybir.AxisListType.X
    Copy = mybir.ActivationFunctionType.Copy
    Exp = mybir.ActivationFunctionType.Exp

    with tile.TileContext(nc) as tc:
        with (
            tc.tile_pool(name="const", bufs=1) as cpool,
            tc.tile_pool(name="slab", bufs=2) as spool,
            tc.tile_pool(name="scratch", bufs=2) as tpool,
            tc.tile_pool(name="ps_bc", bufs=2, space="PSUM") as ppool,
            tc.tile_pool(name="ps_sm", bufs=4, space="PSUM") as qpool,
        ):
            # ---- streaming-side DMAs (sync queue): h-broadcast first, then
            # the enc slabs keep the queue saturated ----
            hbext = cpool.tile([P, BLOC * HE], F32)
            nc.sync.dma_start(hbext[:], hbx_d[:])

            # ---- small inputs on the gpsimd DMA queue ----
            h_sb = cpool.tile([BLOC, H], F32)
            nc.gpsimd.dma_start(h_sb[:], hid_d[:])
            # affT_sb[p, ho*A+k] = affect[ho*128+p, k] — h lands on partitions
            affT_sb = cpool.tile([P, (H // P) * A], F32)
            nc.gpsimd.dma_start(
                affT_sb[:], aff_d[:].rearrange("(ho p) k -> p ho k", p=P))
            ident = cpool.tile([P, P], F32)
            nc.gpsimd.dma_start(ident[:], ident_d[:])
            ones = cpool.tile([1, P], F32)
            nc.gpsimd.dma_start(ones[:], ones_d[:])
            sel = cpool.tile([BLOC, BLOC * P], F32)
            nc.gpsimd.dma_start(sel[:], sel_d[:])
            bm = cpool.tile([cols, BLOC], F32)
            nc.gpsimd.dma_start(bm[:], bm_d[:])
            bmT = cpool.tile([BLOC, cols], F32)
            nc.gpsimd.dma_start(bmT[:], bmT_d[:])
            nbmT = cpool.tile([BLOC, cols], F32)
            nc.gpsimd.dma_start(nbmT[:], nbmT_d[:])

            # ---- ha = h @ affect ([BLOC, A]) on the TensorEngine: transpose
            # h into [h-part, b] blocks, accumulate K=128 matmuls against the
            # h-partitioned affect tile ----
            nho = H // P
            hT_sb = cpool.tile([P, nho * BLOC], F32)
            for ho in range(nho):
                hT_ps = ppool.tile([P, BLOC], F32, tag="bc", name="hT_ps")
                nc.tensor.transpose(hT_ps[:], h_sb[:, bass.ts(ho, P)],
                                    ident[0:BLOC, 0:BLOC])
                nc.vector.tensor_copy(hT_sb[:, bass.ts(ho, BLOC)], hT_ps[:])
            ha_ps = ppool.tile([BLOC, A], F32, tag="bc", name="ha_ps")
            for ho in range(nho):
                nc.tensor.matmul(
                    ha_ps[:],
                    hT_sb[:, bass.ts(ho, BLOC)],
                    affT_sb[:, bass.ts(ho, A)],
                    start=(ho == 0), stop=(ho == nho - 1),
                )
            ha_sb = cpool.tile([BLOC, A], F32)
            nc.vector.tensor_copy(ha_sb[:], ha_ps[:])

            # fill the per-batch ha columns of hbext: one selector matmul per
            # b into a [P, 8*A] psum tile, then a single strided copy
            habx_ps = ppool.tile([P, BLOC * A], F32, tag="bc", name="habx_ps")
            for b in range(BLOC):
                nc.tensor.matmul(habx_ps[:, b * A:(b + 1) * A],
                                 sel[:, bass.ts(b, P)], ha_sb[:],
                                 start=True, stop=True)
            nc.vector.tensor_copy(
                hbext[:].rearrange("p (b f) -> p b f", b=BLOC)[:, :, H:HE],
                habx_ps[:].rearrange("p (b k) -> p b k", b=BLOC),
            )

            # ---- main loop: stream enc slabs; one wide VectorE multiply per
            # slab; reductions split 1/7 VectorE/ScalarE; scores transposed
            # incrementally (o-major columns) ----
            scores = cpool.tile([P, cols], F32)
            scT = cpool.tile([cols, P], F32)
            enc_r = enc_d[:].rearrange("(o p) f -> o p f", p=P)
            for o in range(no):
                slab = spool.tile([P, BLOC * HE], F32, tag="slab", name="slab")
                nc.sync.dma_start(slab[:], enc_r[o])
                prod = tpool.tile([P, BLOC * HE], F32, tag="prod", name="prod")
                nc.vector.tensor_mul(prod[:], slab[:], hbext[:])
                for b in range(BLOC):
                    c = o * BLOC + b
                    pseg = prod[:, b * HE:(b + 1) * HE]
                    if b == 0:
                        nc.vector.tensor_reduce(scores[:, c:c + 1], pseg,
                                                axis=AX, op=add)
                    else:
                        cpy = tpool.tile([P, HE], F32, tag="cpy", name="cpy")
                        nc.scalar.activation(cpy[:], pseg, Copy,
                                             accum_out=scores[:, c:c + 1])
                # transpose score columns into scT rows, 4 slabs (32 cols) at
                # a time — engine partition offsets must be multiples of 32
                if (o + 1) % 4 == 0 or o == no - 1:
                    gs = (o // 4) * 4 * BLOC          # first column of group
                    gw = (o + 1) * BLOC - gs          # columns in group
                    stp = qpool.tile([32, P], F32, tag="sm", name="stp")
                    nc.tensor.transpose(stp[0:gw, :], scores[:, gs:gs + gw],
                                        ident[:])
                    nc.scalar.copy(scT[gs:gs + gw, :], stp[0:gw, :])

            # ---- softmax tail on scT[c = o*8+b, li] ----
            rowmax = cpool.tile([cols, 1], F32)
            nc.vector.tensor_reduce(rowmax[:], scT[:], axis=AX, op=amax)
            rmT_ps = qpool.tile([1, cols], F32, tag="sm", name="rmT_ps")
            nc.tensor.matmul(rmT_ps[:], rowmax[:], ident[0:cols, 0:cols],
                             start=True, stop=True)
            rm_sb = cpool.tile([1, cols], F32)
            nc.scalar.copy(rm_sb[:], rmT_ps[:])
            bmax = cpool.tile([1, BLOC], F32)
            nc.vector.tensor_reduce(
                bmax[:], rm_sb[:].rearrange("p (o b) -> p b o", b=BLOC),
                axis=AX, op=amax)
            bcol_ps = qpool.tile([BLOC, 1], F32, tag="sm", name="bcol_ps")
            nc.tensor.matmul(bcol_ps[:], bmax[:], ones[0:1, 0:1],
                             start=True, stop=True)
            bcol = cpool.tile([BLOC, 1], F32)
            nc.scalar.copy(bcol[:], bcol_ps[:])
            negm_ps = qpool.tile([cols, 1], F32, tag="sm", name="negm_ps")
            nc.tensor.matmul(negm_ps[:], nbmT[:], bcol[:], start=True, stop=True)
            negm = cpool.tile([cols, 1], F32)
            nc.scalar.copy(negm[:], negm_ps[:])

            expT = cpool.tile([cols, P], F32)
            rowsum = cpool.tile([cols, 1], F32)
            nc.scalar.activation(expT[:], scT[:], Exp, bias=negm[:], scale=1.0,
                                 accum_out=rowsum[:])
            ssum_ps = qpool.tile([BLOC, 1], F32, tag="sm", name="ssum_ps")
            nc.tensor.matmul(ssum_ps[:], bm[:], rowsum[:], start=True, stop=True)
            rsum = cpool.tile([BLOC, 1], F32)
            nc.vector.reciprocal(rsum[:], ssum_ps[:])
            rbc_ps = qpool.tile([cols, 1], F32, tag="sm", name="rbc_ps")
            nc.tensor.matmul(rbc_ps[:], bmT[:], rsum[:], start=True, stop=True)
            rbc = cpool.tile([cols, 1], F32)
            nc.scalar.copy(rbc[:], rbc_ps[:])

            outT = cpool.tile([cols, P], F32)
            nc.vector.tensor_scalar_mul(outT[:], expT[:], rbc[:, 0:1])
            # out is o-major [cols, P]; the host un-permutes rows
            nc.sync.dma_start(out_d[:], outT[:])

    nc.compile()
    return nc


def make_aux(l_total: int = L):
    no = l_total // P
    cols = BLOC * no
    ident = np.eye(P, dtype=np.float32)
    ones_ = np.ones((1, P), dtype=np.float32)
    # o-major: column c = o*BLOC + b belongs to batch b = c % BLOC
    bmT = np.zeros((BLOC, cols), dtype=np.float32)
    for b in range(BLOC):
        bmT[b, b::BLOC] = 1.0
    sel = np.zeros((BLOC, BLOC * P), dtype=np.float32)
    for b in range(BLOC):
        sel[b, b * P:(b + 1) * P] = 1.0
    return {
        "ident": ident,
        "ones_": ones_,
        "bm": np.ascontiguousarray(bmT.T),
        "bmT": bmT,
        "nbmT": -bmT,
        "sel": sel,
    }


def make_in_maps(hidden, encoder_outputs, embedding, affect_matrix, l_total: int = L):
    aux = make_aux(l_total)
    aff = np.ascontiguousarray(affect_matrix, dtype=np.float32)
    in_maps = []
    for i in range(NCORES):
        bs = slice(i * BLOC, (i + 1) * BLOC)
        enc_ext = np.concatenate(
            [encoder_outputs[:, bs, :], embedding[:, bs, :]], axis=2
        ).reshape(l_total, BLOC * HE)
        hid_loc = np.ascontiguousarray(hidden[0, bs, :], dtype=np.float32)
        hbx = np.zeros((P, BLOC * HE), dtype=np.float32)
        for b in range(BLOC):
            hbx[:, b * HE:b * HE + H] = hid_loc[b]
        in_maps.append({
            "enc": np.ascontiguousarray(enc_ext, dtype=np.float32),
            "hid": hid_loc,
            "aff": aff,
            "hbx": hbx,
            **aux,
        })
    return in_maps


_NC_CACHE = {}


def kernel(hidden, encoder_outputs, embedding, affect_matrix):
    hidden = np.asarray(hidden, dtype=np.float32)
    encoder_outputs = np.asarray(encoder_outputs, dtype=np.float32)
    embedding = np.asarray(embedding, dtype=np.float32)
    affect_matrix = np.asarray(affect_matrix, dtype=np.float32)

    if L not in _NC_CACHE:
        _NC_CACHE[L] = build_nc(L)
    nc = _NC_CACHE[L]
    in_maps = make_in_maps(hidden, encoder_outputs, embedding, affect_matrix, L)
    res = run_bass_kernel_spmd(nc, in_maps, list(range(NCORES))).results
    no = L // P
    out = np.concatenate(
        [res[i]["out"].reshape(no, BLOC, P).transpose(1, 0, 2).reshape(BLOC, 1, L)
         for i in range(NCORES)],
        axis=0,
    )
    return out



# revision 7
# speedup vs baseline: 1.8126x; 1.8126x over previous
"""Bass/Trainium2 kernel for nn_Attn (dot+affect attention over encoder outputs).

Computation (per batch b):
  e[b, l] = h[b] . enc[l, b]  +  (h[b] @ affect) . emb[l, b]
  out[b, 0, :] = softmax(e[b, :])

Strategy: data-parallel over batch (8 batches per core on 8 cores). The host
casts enc/emb/h to fp16 and pre-transposes enc so the hidden (contraction) dim
lies on SBUF partitions. The TensorEngine then computes the attention energies
directly: per (batch, 512-wide l-chunk), eight K=128 matmuls (stationary
hT[128,8], moving enc chunk [128,512]) plus one K=3 matmul for the affect term
(haT = affT @ hT computed on-device) accumulate f32 scores in PSUM. One psum
row per chunk is the true per-batch score segment; it is copied into a
row-major [8, 2048] scores tile, so the softmax tail is three short
whole-tile ops with no transposes. DMA (fp16, ~33.6 MB/core) is the bottleneck;
PE runs at ~60-70% occupancy underneath it and DVE/ACT are nearly idle.
"""

import numpy as np

import concourse.bass as bass
import concourse.tile as tile
from concourse import bacc, mybir
from concourse.bass_utils import run_bass_kernel_spmd

F32 = mybir.dt.float32
F16 = mybir.dt.float16
L, B, H, A = 2048, 64, 1024, 3
NCORES = 8
BLOC = B // NCORES          # batches per core
P = 128                     # SBUF partitions
CH = 512                    # l-chunk width (one psum bank of f32)
NC_CH = L // CH             # chunks per batch (4)
NHO = H // P                # h-blocks (8)
NGRP = BLOC * NC_CH         # (b, c) groups per core (32)
SLAB = 8192                 # stream slab width (2 groups)
NO = (NGRP * NHO * CH) // SLAB  # slabs (16)


def build_nc():
    nc = bacc.Bacc("TRN2", target_bir_lowering=False, debug=False)

    enc_d = nc.dram_tensor("enc", [P, NO * SLAB], F16, kind="ExternalInput")
    emb_d = nc.dram_tensor("emb", [A, NGRP * CH], F16, kind="ExternalInput")
    ht_d = nc.dram_tensor("ht", [P, NHO * BLOC], F16, kind="ExternalInput")
    afft_d = nc.dram_tensor("afft", [P, NHO * A], F16, kind="ExternalInput")
    mneg_d = nc.dram_tensor("mneg", [BLOC, NGRP], F32, kind="ExternalInput")
    out_d = nc.dram_tensor("out", [BLOC, L], F32, kind="ExternalOutput")

    add = mybir.AluOpType.add
    amax = mybir.AluOpType.max
    AX = mybir.AxisListType.X
    Exp = mybir.ActivationFunctionType.Exp

    with tile.TileContext(nc) as tc:
        with (
            tc.tile_pool(name="const", bufs=1) as cpool,
            tc.tile_pool(name="slab", bufs=3) as spool,
            tc.tile_pool(name="ps", bufs=6, space="PSUM") as ppool,
            tc.tile_pool(name="ps_ha", bufs=1, space="PSUM") as hpool,
        ):
            # small inputs on the gpsimd (SWDGE) queue; the big enc stream
            # owns the sync HWDGE queue from t=0
            ht = cpool.tile([P, NHO * BLOC], F16)
            nc.gpsimd.dma_start(ht[:], ht_d[:])
            afft = cpool.tile([P, NHO * A], F16)
            nc.gpsimd.dma_start(afft[:], afft_d[:])
            embt = cpool.tile([A, NGRP * CH], F16)
            nc.gpsimd.dma_start(embt[:], emb_d[:])
            mneg = cpool.tile([BLOC, NGRP], F32)
            nc.gpsimd.dma_start(mneg[:], mneg_d[:])

            # haT[a, b] = sum_h affect[h, a] * h[b, h]  (K-accumulate over
            # h-blocks; both operands arrive h-on-partitions)
            ha_ps = hpool.tile([A, BLOC], F32, tag="ha", name="ha_ps")
            for ho in range(NHO):
                nc.tensor.matmul(
                    ha_ps[:],
                    afft[:, ho * A:(ho + 1) * A],
                    ht[:, ho * BLOC:(ho + 1) * BLOC],
                    start=(ho == 0), stop=(ho == NHO - 1),
                )
            hat = cpool.tile([A, BLOC], F16)
            nc.vector.tensor_copy(hat[:], ha_ps[:])

            # psum row b of group (b, c) is the true score segment; engine
            # APs need partition offsets % 32 == 0, so whole [8, 512] psum
            # tiles land in a staging tile and per-batch DMAs (free partition
            # addressing) gather the true rows into row-major scores
            staging = cpool.tile([BLOC, NGRP * CH], F32)
            scores = cpool.tile([BLOC, L], F32)
            pmax = cpool.tile([BLOC, NGRP], F32)

            for o in range(NO):
                slab = spool.tile([P, SLAB], F16, tag="slab", name="slab")
                nc.sync.dma_start(slab[:], enc_d[:, o * SLAB:(o + 1) * SLAB])
                for gg in range(2):
                    g = o * 2 + gg            # g = b * NC_CH + c
                    b, c = divmod(g, NC_CH)
                    ps = ppool.tile([BLOC, CH], F32, tag="ps", name="ps")
                    for ho in range(NHO):
                        nc.tensor.matmul(
                            ps[:],
                            ht[:, ho * BLOC:(ho + 1) * BLOC],
                            slab[:, gg * (NHO * CH) + ho * CH:
                                 gg * (NHO * CH) + (ho + 1) * CH],
                            start=(ho == 0), stop=False,
                        )
                    nc.tensor.matmul(
                        ps[:], hat[:], embt[:, g * CH:(g + 1) * CH],
                        start=False, stop=True,
                    )
                    nc.scalar.copy(staging[:, g * CH:(g + 1) * CH], ps[:])
                    nc.vector.tensor_reduce(pmax[:, g:g + 1], ps[:],
                                            axis=AX, op=amax)
                    if c == NC_CH - 1:
                        # batch b complete: gather its 4 true segments
                        nc.gpsimd.dma_start(
                            scores[b:b + 1, :],
                            staging[b:b + 1, b * L:(b + 1) * L])

            # softmax tail, all 8 batch rows at once; mneg (-1e30 on
            # foreign-row entries) masks garbage chunk maxes
            pm2 = cpool.tile([BLOC, NGRP], F32)
            nc.vector.tensor_add(pm2[:], pmax[:], mneg[:])
            rmax = cpool.tile([BLOC, 1], F32)
            nc.vector.tensor_reduce(rmax[:], pm2[:], axis=AX, op=amax)
            nmax = cpool.tile([BLOC, 1], F32)
            nc.vector.tensor_scalar_mul(nmax[:], rmax[:], -1.0)
            ex = cpool.tile([BLOC, L], F32)
            rsum = cpool.tile([BLOC, 1], F32)
            nc.scalar.activation(ex[:], scores[:], Exp, bias=nmax[:],
                                 scale=1.0, accum_out=rsum[:])
            rcp = cpool.tile([BLOC, 1], F32)
            nc.vector.reciprocal(rcp[:], rsum[:])
            outt = cpool.tile([BLOC, L], F32)
            nc.vector.tensor_scalar_mul(outt[:], ex[:], rcp[:, 0:1])
            nc.sync.dma_start(out_d[:], outt[:])

    nc.compile()
    return nc


def make_in_maps(hidden, encoder_outputs, embedding, affect_matrix):
    aff16 = np.ascontiguousarray(affect_matrix, dtype=np.float16)
    # affT[k, ho*A + a] = affect[ho*128 + k, a]
    afft = np.ascontiguousarray(
        aff16.reshape(NHO, P, A).transpose(1, 0, 2).reshape(P, NHO * A))
    # mneg[b, g] = 0 where group g belongs to batch b, else -1e30
    mneg = np.full((BLOC, NGRP), -1e30, dtype=np.float32)
    for b in range(BLOC):
        mneg[b, b * NC_CH:(b + 1) * NC_CH] = 0.0
    in_maps = []
    for i in range(NCORES):
        bs = slice(i * BLOC, (i + 1) * BLOC)
        enc16 = encoder_outputs[:, bs, :].astype(np.float16)  # [L, 8, H]
        # encT[k, (b, c, ho, j)] = enc[c*512 + j, b, ho*128 + k]
        enct = np.ascontiguousarray(
            enc16.reshape(NC_CH, CH, BLOC, NHO, P)
            .transpose(4, 2, 0, 3, 1).reshape(P, NO * SLAB))
        emb16 = embedding[:, bs, :].astype(np.float16)        # [L, 8, A]
        # embT[a, (b, c, j)] = emb[c*512 + j, b, a]
        embt = np.ascontiguousarray(
            emb16.reshape(NC_CH, CH, BLOC, A)
            .transpose(3, 2, 0, 1).reshape(A, NGRP * CH))
        h16 = hidden[0, bs, :].astype(np.float16)             # [8, H]
        # hT[k, ho*BLOC + b] = h[b, ho*128 + k]
        ht = np.ascontiguousarray(
            h16.reshape(BLOC, NHO, P).transpose(2, 1, 0).reshape(P, NHO * BLOC))
        in_maps.append({"enc": enct, "emb": embt, "ht": ht, "afft": afft,
                        "mneg": mneg})
    return in_maps


def assemble(results):
    return np.concatenate(
        [np.asarray(results[i]["out"], dtype=np.float32)[:, None, :]
         for i in range(NCORES)], axis=0)


_NC_CACHE = {}


def kernel(hidden, encoder_outputs, embedding, affect_matrix):
    hidden = np.asarray(hidden, dtype=np.float32)
    encoder_outputs = np.asarray(encoder_outputs, dtype=np.float32)
    embedding = np.asarray(embedding, dtype=np.float32)
    affect_matrix = np.asarray(affect_matrix, dtype=np.float32)

    if "nc" not in _NC_CACHE:
        _NC_CACHE["nc"] = build_nc()
    nc = _NC_CACHE["nc"]
    in_maps = make_in_maps(hidden, encoder_outputs, embedding, affect_matrix)
    res = run_bass_kernel_spmd(nc, in_maps, list(range(NCORES))).results
    return assemble(res)


# revision 8
# speedup vs baseline: 2.0363x; 1.1234x over previous
"""Bass/Trainium2 kernel for nn_Attn (dot+affect attention over encoder outputs).

Computation (per batch b):
  e[b, l] = h[b] . enc[l, b]  +  (h[b] @ affect) . emb[l, b]
  out[b, 0, :] = softmax(e[b, :])

Strategy: data-parallel over batch (8 batches per core on 8 cores). The host
casts enc/emb/h to fp16 and pre-transposes enc so the hidden (contraction) dim
lies on SBUF partitions. The TensorEngine then computes the attention energies
directly: per (batch, 512-wide l-chunk) group, eight K=128 matmuls (stationary
hT[128,8], moving enc chunk [128,512]) plus one K=3 matmul for the affect term
(haT = affT @ hT computed on-device) accumulate f32 scores in PSUM. Each group
has its own 1MB DMA so the PE trails the stream by at most one group. The
softmax runs online: each batch's chunk-0 max is the exp bias for all its
chunks (f32 absorbs the range), so ScalarE's psum->SBUF copy is already the
exp pass and the tail is just a masked row-sum + reciprocal + normalize.
Engine APs need partition offsets % 32 == 0, so whole [8, 512] psum tiles land
in a staging tile (true scores in row b, cross-batch garbage elsewhere) and
per-batch DMAs (free partition addressing) gather the true rows. DMA
(fp16, ~33.6 MB/core) is the bottleneck; PE runs at ~65% occupancy underneath
it and DVE is nearly idle.
"""

import numpy as np

import concourse.bass as bass
import concourse.tile as tile
from concourse import bacc, mybir
from concourse.bass_utils import run_bass_kernel_spmd

F32 = mybir.dt.float32
F16 = mybir.dt.float16
L, B, H, A = 2048, 64, 1024, 3
NCORES = 8
BLOC = B // NCORES          # batches per core
P = 128                     # SBUF partitions
CH = 512                    # l-chunk width (one psum bank of f32)
NC_CH = L // CH             # chunks per batch (4)
NHO = H // P                # h-blocks (8)
NGRP = BLOC * NC_CH         # (b, c) groups per core (32)
GRPW = NHO * CH             # stream columns per group (4096)


def build_nc():
    nc = bacc.Bacc("TRN2", target_bir_lowering=False, debug=False)

    enc_d = nc.dram_tensor("enc", [P, NGRP * GRPW], F16, kind="ExternalInput")
    emb_d = nc.dram_tensor("emb", [A, NGRP * CH], F16, kind="ExternalInput")
    ht_d = nc.dram_tensor("ht", [P, NHO * BLOC], F16, kind="ExternalInput")
    afft_d = nc.dram_tensor("afft", [P, NHO * A], F16, kind="ExternalInput")
    m01_d = nc.dram_tensor("m01", [BLOC, NGRP], F32, kind="ExternalInput")
    out_d = nc.dram_tensor("out", [BLOC, L], F32, kind="ExternalOutput")

    add = mybir.AluOpType.add
    amax = mybir.AluOpType.max
    AX = mybir.AxisListType.X
    Exp = mybir.ActivationFunctionType.Exp

    with tile.TileContext(nc) as tc:
        with (
            tc.tile_pool(name="const", bufs=1) as cpool,
            tc.tile_pool(name="slab", bufs=6) as spool,
            tc.tile_pool(name="ps", bufs=6, space="PSUM") as ppool,
            tc.tile_pool(name="ps_ha", bufs=1, space="PSUM") as hpool,
        ):
            # small inputs on the gpsimd (SWDGE) queue; the big enc stream
            # owns the sync HWDGE queue from t=0
            ht = cpool.tile([P, NHO * BLOC], F16)
            nc.gpsimd.dma_start(ht[:], ht_d[:])
            afft = cpool.tile([P, NHO * A], F16)
            nc.gpsimd.dma_start(afft[:], afft_d[:])
            embt = cpool.tile([A, NGRP * CH], F16)
            nc.gpsimd.dma_start(embt[:], emb_d[:])
            m01 = cpool.tile([BLOC, NGRP], F32)
            nc.gpsimd.dma_start(m01[:], m01_d[:])

            # haT[a, b] = sum_h affect[h, a] * h[b, h]  (K-accumulate over
            # h-blocks; both operands arrive h-on-partitions)
            ha_ps = hpool.tile([A, BLOC], F32, tag="ha", name="ha_ps")
            for ho in range(NHO):
                nc.tensor.matmul(
                    ha_ps[:],
                    afft[:, ho * A:(ho + 1) * A],
                    ht[:, ho * BLOC:(ho + 1) * BLOC],
                    start=(ho == 0), stop=(ho == NHO - 1),
                )
            hat = cpool.tile([A, BLOC], F16)
            nc.vector.tensor_copy(hat[:], ha_ps[:])

            staging = cpool.tile([BLOC, NGRP * CH], F32)  # exp(e - m0[b])
            scores = cpool.tile([BLOC, L], F32)           # gathered true rows
            pm0 = cpool.tile([BLOC, BLOC], F32)           # chunk-0 maxes
            nm = cpool.tile([BLOC, BLOC], F32)            # -chunk-0 maxes
            cs = cpool.tile([BLOC, NGRP], F32)            # per-group exp sums

            for g in range(NGRP):                         # g = b * NC_CH + c
                b, c = divmod(g, NC_CH)
                slab = spool.tile([P, GRPW], F16, tag="slab", name="slab")
                nc.sync.dma_start(slab[:], enc_d[:, g * GRPW:(g + 1) * GRPW])
                ps = ppool.tile([BLOC, CH], F32, tag="ps", name="ps")
                for ho in range(NHO):
                    nc.tensor.matmul(
                        ps[:],
                        ht[:, ho * BLOC:(ho + 1) * BLOC],
                        slab[:, ho * CH:(ho + 1) * CH],
                        start=(ho == 0), stop=False,
                    )
                nc.tensor.matmul(
                    ps[:], hat[:], embt[:, g * CH:(g + 1) * CH],
                    start=False, stop=True,
                )
                if c == 0:
                    # batch b's exp bias for all four chunks; f32 absorbs
                    # exp(max_c - max_0) comfortably
                    nc.vector.tensor_reduce(pm0[:, b:b + 1], ps[:],
                                            axis=AX, op=amax)
                    nc.vector.tensor_scalar_mul(nm[:, b:b + 1],
                                                pm0[:, b:b + 1], -1.0)
                nc.scalar.activation(staging[:, g * CH:(g + 1) * CH], ps[:],
                                     Exp, bias=nm[:, b:b + 1], scale=1.0,
                                     accum_out=cs[:, g:g + 1])
                if c == NC_CH - 1:
                    # batch b complete: gather its 4 true exp segments
                    nc.gpsimd.dma_start(
                        scores[b:b + 1, :],
                        staging[b:b + 1, b * L:(b + 1) * L])

            # tail: masked row-sum of exp sums (m01 zeroes foreign-row
            # garbage), reciprocal, normalize; output DMA split to overlap
            csm = cpool.tile([BLOC, NGRP], F32)
            nc.vector.tensor_mul(csm[:], cs[:], m01[:])
            ssum = cpool.tile([BLOC, 1], F32)
            nc.vector.tensor_reduce(ssum[:], csm[:], axis=AX, op=add)
            rcp = cpool.tile([BLOC, 1], F32)
            nc.vector.reciprocal(rcp[:], ssum[:])
            outt = cpool.tile([BLOC, L], F32)
            HL = L // 2
            for h in range(2):
                nc.vector.tensor_scalar_mul(outt[:, h * HL:(h + 1) * HL],
                                            scores[:, h * HL:(h + 1) * HL],
                                            rcp[:, 0:1])
                nc.sync.dma_start(out_d[:, h * HL:(h + 1) * HL],
                                  outt[:, h * HL:(h + 1) * HL])

    nc.compile()
    return nc


def make_in_maps(hidden, encoder_outputs, embedding, affect_matrix):
    aff16 = np.ascontiguousarray(affect_matrix, dtype=np.float16)
    # affT[k, ho*A + a] = affect[ho*128 + k, a]
    afft = np.ascontiguousarray(
        aff16.reshape(NHO, P, A).transpose(1, 0, 2).reshape(P, NHO * A))
    # m01[b, g] = 1 where group g belongs to batch b, else 0
    m01 = np.zeros((BLOC, NGRP), dtype=np.float32)
    for b in range(BLOC):
        m01[b, b * NC_CH:(b + 1) * NC_CH] = 1.0
    in_maps = []
    for i in range(NCORES):
        bs = slice(i * BLOC, (i + 1) * BLOC)
        enc16 = encoder_outputs[:, bs, :].astype(np.float16)  # [L, 8, H]
        # encT[k, (b, c, ho, j)] = enc[c*512 + j, b, ho*128 + k]
        enct = np.ascontiguousarray(
            enc16.reshape(NC_CH, CH, BLOC, NHO, P)
            .transpose(4, 2, 0, 3, 1).reshape(P, NGRP * GRPW))
        emb16 = embedding[:, bs, :].astype(np.float16)        # [L, 8, A]
        # embT[a, (b, c, j)] = emb[c*512 + j, b, a]
        embt = np.ascontiguousarray(
            emb16.reshape(NC_CH, CH, BLOC, A)
            .transpose(3, 2, 0, 1).reshape(A, NGRP * CH))
        h16 = hidden[0, bs, :].astype(np.float16)             # [8, H]
        # hT[k, ho*BLOC + b] = h[b, ho*128 + k]
        ht = np.ascontiguousarray(
            h16.reshape(BLOC, NHO, P).transpose(2, 1, 0).reshape(P, NHO * BLOC))
        in_maps.append({"enc": enct, "emb": embt, "ht": ht, "afft": afft,
                        "m01": m01})
    return in_maps


def assemble(results):
    return np.concatenate(
        [np.asarray(results[i]["out"], dtype=np.float32)[:, None, :]
         for i in range(NCORES)], axis=0)


_NC_CACHE = {}


def kernel(hidden, encoder_outputs, embedding, affect_matrix):
    hidden = np.asarray(hidden, dtype=np.float32)
    encoder_outputs = np.asarray(encoder_outputs, dtype=np.float32)
    embedding = np.asarray(embedding, dtype=np.float32)
    affect_matrix = np.asarray(affect_matrix, dtype=np.float32)

    if "nc" not in _NC_CACHE:
        _NC_CACHE["nc"] = build_nc()
    nc = _NC_CACHE["nc"]
    in_maps = make_in_maps(hidden, encoder_outputs, embedding, affect_matrix)
    res = run_bass_kernel_spmd(nc, in_maps, list(range(NCORES))).results
    return assemble(res)


# revision 14
# speedup vs baseline: 2.1082x; 1.0353x over previous
"""Bass/Trainium2 kernel for nn_Attn (dot+affect attention over encoder outputs).

Computation (per batch b):
  e[b, l] = h[b] . enc[l, b]  +  (h[b] @ affect) . emb[l, b]
  out[b, 0, :] = softmax(e[b, :])

Strategy: data-parallel over batch (8 batches per core on 8 cores). The host
casts enc/emb/h to fp16 and pre-transposes enc so the hidden (contraction) dim
lies on SBUF partitions. The TensorEngine then computes the attention energies
directly: per (batch, 512-wide l-chunk) group, eight K=128 matmuls (stationary
hT[128,8], moving enc chunk [128,512]) plus one K=3 matmul for the affect term
(haT = affT @ hT computed on-device) accumulate f32 scores in PSUM. Each group
has its own 1MB DMA so the PE trails the stream by at most one group. The
softmax runs online: each batch's chunk-0 max is the exp bias for all its
chunks (f32 absorbs the range), so ScalarE's psum->SBUF copy is already the
exp pass and the tail is just a masked row-sum + reciprocal + normalize.
Engine APs need partition offsets % 32 == 0, so whole [8, 512] psum tiles land
in a staging tile (true scores in row b, cross-batch garbage elsewhere) and
per-batch DMAs (free partition addressing) gather the true rows. DMA
(fp16, ~33.6 MB/core) is the bottleneck; PE runs at ~65% occupancy underneath
it and DVE is nearly idle.
"""

import numpy as np

import concourse.bass as bass
import concourse.tile as tile
from concourse import bacc, mybir
from concourse.bass_utils import run_bass_kernel_spmd

F32 = mybir.dt.float32
F16 = mybir.dt.float16
L, B, H, A = 2048, 64, 1024, 3
NCORES = 8
BLOC = B // NCORES          # batches per core
P = 128                     # SBUF partitions
CH = 512                    # l-chunk width (one psum bank of f32)
NC_CH = L // CH             # chunks per batch (4)
NHO = H // P                # h-blocks (8)
NGRP = BLOC * NC_CH         # (b, c) groups per core (32)
GRPW = NHO * CH             # stream columns per group (4096)


def build_nc():
    nc = bacc.Bacc("TRN2", target_bir_lowering=False, debug=False)

    enc_d = nc.dram_tensor("enc", [P, NGRP * GRPW], F16, kind="ExternalInput")
    emb_d = nc.dram_tensor("emb", [A, NGRP * CH], F16, kind="ExternalInput")
    ht_d = nc.dram_tensor("ht", [P, NHO * BLOC], F16, kind="ExternalInput")
    afft_d = nc.dram_tensor("afft", [P, NHO * A], F16, kind="ExternalInput")
    out_d = nc.dram_tensor("out", [BLOC, L], F32, kind="ExternalOutput")

    add = mybir.AluOpType.add
    amax = mybir.AluOpType.max
    AX = mybir.AxisListType.X
    Exp = mybir.ActivationFunctionType.Exp

    with tile.TileContext(nc) as tc:
        with (
            tc.tile_pool(name="const", bufs=1) as cpool,
            tc.tile_pool(name="slab", bufs=5) as spool,
            tc.tile_pool(name="ps", bufs=6, space="PSUM") as ppool,
            tc.tile_pool(name="ps_ha", bufs=1, space="PSUM") as hpool,
        ):
            # small inputs on the gpsimd (SWDGE) queue; the big enc stream
            # owns the sync HWDGE queue from t=0
            ht = cpool.tile([P, NHO * BLOC], F16)
            nc.gpsimd.dma_start(ht[:], ht_d[:])
            afft = cpool.tile([P, NHO * A], F16)
            nc.gpsimd.dma_start(afft[:], afft_d[:])
            embt = cpool.tile([A, NGRP * CH], F16)
            nc.gpsimd.dma_start(embt[:], emb_d[:])

            # haT[a, b] = sum_h affect[h, a] * h[b, h]  (K-accumulate over
            # h-blocks; both operands arrive h-on-partitions)
            ha_ps = hpool.tile([A, BLOC], F32, tag="ha", name="ha_ps")
            for ho in range(NHO):
                nc.tensor.matmul(
                    ha_ps[:],
                    afft[:, ho * A:(ho + 1) * A],
                    ht[:, ho * BLOC:(ho + 1) * BLOC],
                    start=(ho == 0), stop=(ho == NHO - 1),
                )
            hat = cpool.tile([A, BLOC], F16)
            nc.vector.tensor_copy(hat[:], ha_ps[:])

            staging = cpool.tile([BLOC, NGRP * CH], F32)  # exp(e - m0[b])
            outstg = cpool.tile([BLOC, NGRP * CH], F32)   # normalized
            pm0 = cpool.tile([BLOC, BLOC], F32)           # chunk-0 maxes
            nm = cpool.tile([BLOC, BLOC], F32)            # -chunk-0 maxes
            cs = cpool.tile([BLOC, NGRP], F32)            # per-group exp sums
            sums = cpool.tile([BLOC, BLOC], F32)          # per-batch exp sums
            rc = cpool.tile([BLOC, BLOC], F32)            # reciprocals
            HG = GRPW // 2

            for g in range(NGRP):                         # g = b * NC_CH + c
                b, c = divmod(g, NC_CH)
                slab = spool.tile([P, GRPW], F16, tag="slab", name="slab")
                # two half-group DMAs so the first 4 matmuls start earlier
                # and the PE trails the stream by only half a group
                nc.sync.dma_start(slab[:, 0:HG],
                                  enc_d[:, g * GRPW:g * GRPW + HG])
                nc.sync.dma_start(slab[:, HG:GRPW],
                                  enc_d[:, g * GRPW + HG:(g + 1) * GRPW])
                ps = ppool.tile([BLOC, CH], F32, tag="ps", name="ps")
                for ho in range(NHO):
                    nc.tensor.matmul(
                        ps[:],
                        ht[:, ho * BLOC:(ho + 1) * BLOC],
                        slab[:, ho * CH:(ho + 1) * CH],
                        start=(ho == 0), stop=False,
                    )
                nc.tensor.matmul(
                    ps[:], hat[:], embt[:, g * CH:(g + 1) * CH],
                    start=False, stop=True,
                )
                if c == 0:
                    # batch b's exp bias for all four chunks; f32 absorbs
                    # exp(max_c - max_0) comfortably
                    nc.vector.tensor_reduce(pm0[:, b:b + 1], ps[:],
                                            axis=AX, op=amax)
                    nc.vector.tensor_scalar_mul(nm[:, b:b + 1],
                                                pm0[:, b:b + 1], -1.0)
                nc.scalar.activation(staging[:, g * CH:(g + 1) * CH], ps[:],
                                     Exp, bias=nm[:, b:b + 1], scale=1.0,
                                     accum_out=cs[:, g:g + 1])
                if c == NC_CH - 1:
                    # batch b complete: row-sum its 4 exp sums (foreign rows
                    # give garbage reciprocals applied only to garbage
                    # entries), normalize its staging columns split across
                    # DVE and ACT, and DMA the true row straight to HBM
                    nc.vector.tensor_reduce(
                        sums[:, b:b + 1], cs[:, b * NC_CH:(b + 1) * NC_CH],
                        axis=AX, op=add)
                    nc.vector.reciprocal(rc[:, b:b + 1], sums[:, b:b + 1])
                    lo = b * L
                    nc.vector.tensor_scalar_mul(
                        outstg[:, lo:lo + L // 2],
                        staging[:, lo:lo + L // 2], rc[:, b:b + 1])
                    nc.scalar.mul(
                        outstg[:, lo + L // 2:lo + L],
                        staging[:, lo + L // 2:lo + L], rc[:, b:b + 1])
                    nc.gpsimd.dma_start(
                        out_d[b:b + 1, 0:L // 2],
                        outstg[b:b + 1, lo:lo + L // 2])
                    nc.gpsimd.dma_start(
                        out_d[b:b + 1, L // 2:L],
                        outstg[b:b + 1, lo + L // 2:lo + L])

    nc.compile()
    return nc


def make_in_maps(hidden, encoder_outputs, embedding, affect_matrix):
    aff16 = np.ascontiguousarray(affect_matrix, dtype=np.float16)
    # affT[k, ho*A + a] = affect[ho*128 + k, a]
    afft = np.ascontiguousarray(
        aff16.reshape(NHO, P, A).transpose(1, 0, 2).reshape(P, NHO * A))
    in_maps = []
    for i in range(NCORES):
        bs = slice(i * BLOC, (i + 1) * BLOC)
        enc16 = encoder_outputs[:, bs, :].astype(np.float16)  # [L, 8, H]
        # encT[k, (b, c, ho, j)] = enc[c*512 + j, b, ho*128 + k]
        enct = np.ascontiguousarray(
            enc16.reshape(NC_CH, CH, BLOC, NHO, P)
            .transpose(4, 2, 0, 3, 1).reshape(P, NGRP * GRPW))
        emb16 = embedding[:, bs, :].astype(np.float16)        # [L, 8, A]
        # embT[a, (b, c, j)] = emb[c*512 + j, b, a]
        embt = np.ascontiguousarray(
            emb16.reshape(NC_CH, CH, BLOC, A)
            .transpose(3, 2, 0, 1).reshape(A, NGRP * CH))
        h16 = hidden[0, bs, :].astype(np.float16)             # [8, H]
        # hT[k, ho*BLOC + b] = h[b, ho*128 + k]
        ht = np.ascontiguousarray(
            h16.reshape(BLOC, NHO, P).transpose(2, 1, 0).reshape(P, NHO * BLOC))
        in_maps.append({"enc": enct, "emb": embt, "ht": ht, "afft": afft})
    return in_maps


def assemble(results):
    return np.concatenate(
        [np.asarray(results[i]["out"], dtype=np.float32)[:, None, :]
         for i in range(NCORES)], axis=0)


_NC_CACHE = {}


def kernel(hidden, encoder_outputs, embedding, affect_matrix):
    hidden = np.asarray(hidden, dtype=np.float32)
    encoder_outputs = np.asarray(encoder_outputs, dtype=np.float32)
    embedding = np.asarray(embedding, dtype=np.float32)
    affect_matrix = np.asarray(affect_matrix, dtype=np.float32)

    if "nc" not in _NC_CACHE:
        _NC_CACHE["nc"] = build_nc()
    nc = _NC_CACHE["nc"]
    in_maps = make_in_maps(hidden, encoder_outputs, embedding, affect_matrix)
    res = run_bass_kernel_spmd(nc, in_maps, list(range(NCORES))).results
    return assemble(res)
